# revision 1
# baseline (speedup 1.0000x reference)
"""Trainium2 Bass kernel for the per-cell star-graph GAT encoder.

Math: the reference returns only the anchor-node (node 0) output of a 1-layer
GAT over a (T+1)-node graph per cell. The anchor's adjacency row contains only
the star edges (anchor self-loop + all valid transcripts), so the kNN graph is
dead code for the output. With h_anchor = 0 the output reduces, per cell, to

    s_j    = tx_feat_j . (W_gat @ a_src)            (per transcript scalar)
    l_j    = leaky_relu(s_j, 0.2)  (+ -1e30 where invalid)
    e_j    = exp(l_j)        den = 1 + sum_j e_j    (anchor logit is 0)
    pooled = sum_j e_j tx_feat_j / den              (35-dim)
    out    = pooled @ W_gat + b_gat, zeroed where cell inactive

where tx_feat_j = [rel_xy (2) | gene_emb[id_j] (32) | qv (1)].

Sharding: data-parallel over the 1024 = B*Q cells, 128 cells per NeuronCore.
Per core, cells sit on SBUF partitions; the gene table (with the per-gene
scalar gs = gene_emb @ wa[2:34] prepended as column 0, padded to 256B rows)
is gathered from DRAM by dma_gather in 6 tapering t-chunks of 4 quarter ops
each, spread over the 4 SWDGE queues (pattern 1,2,3,0: three async queues
dispatch instantly, queue 0 holds the engine while all four generate
descriptors concurrently), pipelined against the DVE/ACT compute.
"""

import numpy as np

import concourse.bacc as bacc
import concourse.mybir as mybir
import concourse.tile as tile
from concourse.bass_utils import run_bass_kernel_spmd
from concourse.tile import add_dep_helper

F32 = mybir.dt.float32
I32 = mybir.dt.int32
AX = mybir.AxisListType
OP = mybir.AluOpType
AF = mybir.ActivationFunctionType

N_CORES = 8
B, Q, T = 4, 256, 128
CELLS = (B * Q) // N_CORES  # 128 cells per core
GENE_VOCAB = 20000
GENE_DIM = 32
F = 64  # padded table row: [gs | gene_emb(32) | zeros], 256B for dma_gather
IN_DIM = 35
D = 256
NEG_SLOPE = 0.2
NCH = 6
CHUNK_T = (32, 28, 24, 20, 16, 8)  # big early, tiny tail round
assert sum(CHUNK_T) == T

_CACHE = {}


def build_program(dbg=False):
    nc = bacc.Bacc("TRN2", target_bir_lowering=False, debug=False,
                   num_devices=N_CORES, num_swdge_queues=4)
    dbg_tensors = {}
    if dbg:
        for nm, w in [("d_spre", T), ("d_e", T), ("d_pooled", IN_DIM),
                      ("d_den", 1), ("d_lhsT", CELLS),
                      ("d_g0", max(CHUNK_T) * F)]:
            dbg_tensors[nm] = nc.dram_tensor(nm, [128, w], F32,
                                             kind="ExternalOutput")

    I16 = mybir.dt.int16
    x_d = nc.dram_tensor("x_in", [CELLS, T * 2], F32, kind="ExternalInput")
    qv_d = nc.dram_tensor("qv_in", [CELLS, T], F32, kind="ExternalInput")
    madd_d = nc.dram_tensor("madd_in", [CELLS, T], F32, kind="ExternalInput")
    # gene ids as int16, dma_gather wrapping: value for flat index i=(t*128+c)
    # at [i % 16, i // 16], replicated 8x down the 128 partitions
    idx_d = nc.dram_tensor("idx_in", [128, (CELLS * T) // 16], I16,
                           kind="ExternalInput")
    cent_d = nc.dram_tensor("cent_in", [CELLS, 2], F32, kind="ExternalInput")
    act_d = nc.dram_tensor("act_in", [CELLS, 1], F32, kind="ExternalInput")
    # consts rows are identical across partitions:
    #   cols 0:256   = tile([wa0, wa1], T)     (for the rel . wa dot)
    #   cols 256:512 = b_gat
    consts_d = nc.dram_tensor("consts_in", [CELLS, 516], F32, kind="ExternalInput")
    wrhs_d = nc.dram_tensor("wrhs_in", [IN_DIM, D], F32, kind="ExternalInput")
    table_d = nc.dram_tensor("table_in", [GENE_VOCAB, F], F32, kind="ExternalInput")
    ident_d = nc.dram_tensor("ident_in", [128, 128], F32, kind="ExternalInput")
    out_d = nc.dram_tensor("out", [CELLS, D], F32, kind="ExternalOutput")

    from concourse import library_config

    with tile.TileContext(nc) as tc:
        with (
            tc.tile_pool(name="single", bufs=1) as single,
            tc.tile_pool(name="gpool", bufs=NCH) as gpool,
            tc.tile_pool(name="ipool", bufs=NCH) as ipool,
            tc.tile_pool(name="work", bufs=2) as work,
            tc.tile_pool(name="stats", bufs=NCH + 2) as stats,
            tc.tile_pool(name="psum", bufs=2, space="PSUM") as psum,
        ):
            # --- gather pipeline (critical path) ---
            # Q7 descriptor-gen is ~8ns/idx. SWDGE queues 1-3 dispatch async
            # (run concurrently); queue 0 blocks the gpsimd engine for its
            # full desc-gen. So: emit the queue-1..3 quarters of every chunk
            # first (async), then all queue-0 quarters last — all 4 queues
            # generate descriptors concurrently.
            # the mlp-library Q7 IRAM reload (~12us) gates the first gather;
            # it must be the first thing on the gpsimd stream
            nc.gpsimd.load_library(library_config.mlp)
            idx_sb = single.tile([128, (CELLS * T) // 16], mybir.dt.int16)
            nc.sync.dma_start(out=idx_sb[:], in_=idx_d.ap())

            # queue pattern [1,2,3,0] per chunk: the three async queues
            # dispatch instantly, then the sync queue-0 op holds the engine
            # while all four queues' desc-gen runs concurrently. The
            # period-4 pattern also keeps Tile's 8 round-robin DMA-sem lanes
            # queue-consistent.
            g_t = []
            t_base = 0
            for j in range(NCH):
                tc = CHUNK_T[j]
                tq = tc // 4  # transcripts per quarter-gather
                gj = gpool.tile([CELLS, max(CHUNK_T) * F], F32, tag="g")
                g3w = gj[:].rearrange("p (t f) -> p t f", t=max(CHUNK_T), f=F)
                for sub, queue in enumerate((1, 2, 3, 0)):
                    c0 = (CELLS * (t_base + tq * sub)) // 16
                    nc.gpsimd.dma_gather(
                        out_ap=g3w[:, tq * sub:tq * (sub + 1), :],
                        in_ap=table_d.ap(),
                        idxs_ap=idx_sb[:, c0:c0 + (CELLS * tq) // 16],
                        num_idxs=CELLS * tq,
                        num_idxs_reg=CELLS * tq,
                        elem_size=F,
                        single_packet=False,
                        queue_num=queue,
                    )
                g_t.append(gj)
                t_base += tc

            # --- dense input loads ---
            x_sb = single.tile([CELLS, T * 2], F32)
            nc.sync.dma_start(out=x_sb[:], in_=x_d.ap())
            qv_sb = single.tile([CELLS, T], F32)
            nc.sync.dma_start(out=qv_sb[:], in_=qv_d.ap())
            madd_sb = single.tile([CELLS, T], F32)
            nc.sync.dma_start(out=madd_sb[:], in_=madd_d.ap())
            cent_sb = single.tile([CELLS, 2], F32)
            nc.sync.dma_start(out=cent_sb[:], in_=cent_d.ap())
            act_sb = single.tile([CELLS, 1], F32)
            nc.sync.dma_start(out=act_sb[:], in_=act_d.ap())
            consts_sb = single.tile([CELLS, 516], F32)
            nc.sync.dma_start(out=consts_sb[:], in_=consts_d.ap())
            wrhs_sb = single.tile([128, D], F32)
            nc.sync.dma_start(out=wrhs_sb[:IN_DIM, :], in_=wrhs_d.ap())
            # identity from host via the scalar-engine HWDGE ring: keeps the
            # gpsimd stream free so the mlp-library reload starts immediately
            ident = single.tile([128, 128], F32)
            nc.scalar.dma_start(out=ident[:], in_=ident_d.ap())
            # b_gat * active, ready off the critical path for the epilogue
            bact = single.tile([CELLS, D], F32)
            nc.vector.tensor_scalar_mul(bact[:], consts_sb[:, 256:512],
                                        act_sb[:])

            # --- s_pre = (rel . wa01) + qv*wa34 - cent.wa01 + madd  (no gene) ---
            xw = work.tile([CELLS, T * 2], F32)
            nc.vector.tensor_tensor(out=xw[:], in0=x_sb[:], in1=consts_sb[:, 0:256],
                                    op=OP.mult)
            term_x = single.tile([CELLS, T], F32)
            nc.vector.tensor_reduce(
                out=term_x[:],
                in_=xw[:].rearrange("p (t c) -> p t c", t=T, c=2),
                axis=AX.X, op=OP.add)
            cw = stats.tile([CELLS, 2], F32, tag="cw")
            nc.vector.tensor_tensor(out=cw[:], in0=cent_sb[:], in1=consts_sb[:, 0:2],
                                    op=OP.mult)
            c01 = stats.tile([CELLS, 1], F32, tag="c01")
            nc.vector.tensor_reduce(out=c01[:], in_=cw[:], axis=AX.X, op=OP.add)
            negc01 = stats.tile([CELLS, 1], F32, tag="negc01")
            nc.vector.tensor_scalar_mul(negc01[:], c01[:], -1.0)
            qvs = work.tile([CELLS, T], F32, tag="qvs")
            nc.scalar.activation(out=qvs[:], in_=qv_sb[:], func=AF.Identity,
                                 bias=negc01[:], scale=consts_sb[:, 512:513])
            spre0 = work.tile([CELLS, T], F32, tag="spre0")
            nc.vector.tensor_tensor(out=spre0[:], in0=term_x[:], in1=qvs[:], op=OP.add)
            spre = single.tile([CELLS, T], F32)
            nc.vector.tensor_tensor(out=spre[:], in0=spre0[:], in1=madd_sb[:], op=OP.add)

            # --- per-chunk: s -> leaky relu -> exp -> weighted gene partials.
            # exp writes straight into e_all; pooled-x / pooled-qv are one
            # batched pass over e_all at the end.
            e_all = single.tile([CELLS, T], F32, name="e_all", tag="e_all")
            acc_es = None   # [CELLS, 1] running sum of e
            acc_pg = None   # [CELLS, 32] running gene pool
            prev_pg_inst = None
            t_base = 0
            for j in range(NCH):
                tc = CHUNK_T[j]
                cj = slice(t_base, t_base + tc)
                g3 = g_t[j][:].rearrange("p (t f) -> p t f", t=max(CHUNK_T),
                                         f=F)[:, :tc, :]
                s_j = work.tile([CELLS, tc], F32, tag="s")
                s_inst = nc.vector.tensor_tensor(out=s_j[:], in0=spre[:, cj],
                                                 in1=g3[:, :, 0:1], op=OP.add)
                if prev_pg_inst is not None:
                    # keep the DVE stream in chunk order: without this the
                    # scheduler (using the serialized-gather cost model) parks
                    # every eg/pg op after the last chunk's s/l
                    add_dep_helper(s_inst.ins, prev_pg_inst.ins, False,
                                   "chunk-order DVE stream")
                l_j = work.tile([CELLS, tc], F32, tag="l")
                nc.vector.scalar_tensor_tensor(out=l_j[:], in0=s_j[:],
                                               scalar=NEG_SLOPE, in1=s_j[:],
                                               op0=OP.mult, op1=OP.max)
                esum_j = stats.tile([CELLS, 1], F32, tag="esum")
                nc.scalar.activation(out=e_all[:, cj], in_=l_j[:], func=AF.Exp,
                                     accum_out=esum_j[:])
                eg_j = work.tile([CELLS, tc * GENE_DIM], F32, tag="eg")
                nc.vector.tensor_tensor(
                    out=eg_j[:], in0=g3[:, :, 1:1 + GENE_DIM],
                    in1=e_all[:, cj].to_broadcast([CELLS, tc, GENE_DIM]),
                    op=OP.mult)
                pg_j = stats.tile([CELLS, GENE_DIM], F32, tag="pg")
                prev_pg_inst = nc.vector.tensor_reduce(
                    out=pg_j[:],
                    in_=eg_j[:].rearrange("p (t f) -> p f t", t=tc, f=GENE_DIM),
                    axis=AX.X, op=OP.add)
                if acc_es is None:
                    acc_es, acc_pg = esum_j, pg_j
                elif j < NCH - 1:
                    new_es = stats.tile([CELLS, 1], F32, tag="aes")
                    nc.vector.tensor_tensor(out=new_es[:], in0=acc_es[:],
                                            in1=esum_j[:], op=OP.add)
                    new_pg = stats.tile([CELLS, GENE_DIM], F32, tag="apg")
                    nc.vector.tensor_tensor(out=new_pg[:], in0=acc_pg[:],
                                            in1=pg_j[:], op=OP.add)
                    acc_es, acc_pg = new_es, new_pg
                else:
                    # last chunk: fuse final accumulate into den / pooled
                    last_es, last_pg = esum_j, pg_j
                t_base += tc
                if j == NCH - 2:
                    # batched pooled-x / pooled-qv over every chunk but the
                    # last, off the tail (runs while the last round gathers)
                    TM = t_base
                    ex = work.tile([CELLS, TM * 2], F32, tag="exall")
                    nc.vector.tensor_tensor(
                        out=ex[:], in0=x_sb[:, :TM * 2],
                        in1=e_all[:, :TM].to_broadcast([CELLS, TM, 2]),
                        op=OP.mult)
                    px_m = stats.tile([CELLS, 2], F32, tag="pxm")
                    nc.vector.tensor_reduce(
                        out=px_m[:],
                        in_=ex[:].rearrange("p (t c) -> p c t", t=TM, c=2),
                        axis=AX.X, op=OP.add)
                    pqs_m = work.tile([CELLS, TM], F32, tag="pqm")
                    nc.vector.tensor_tensor(out=pqs_m[:], in0=qv_sb[:, :TM],
                                            in1=e_all[:, :TM], op=OP.mult)
                    pq_m = stats.tile([CELLS, 1], F32, tag="pqms")
                    nc.vector.tensor_reduce(out=pq_m[:], in_=pqs_m[:],
                                            axis=AX.X, op=OP.add)

            # --- last-chunk pooled-x / pooled-qv + combine + final matmul ---
            TL = CHUNK_T[-1]
            exl = stats.tile([CELLS, TL * 2], F32, tag="exl")
            nc.vector.tensor_tensor(
                out=exl[:], in0=x_sb[:, (T - TL) * 2:],
                in1=e_all[:, T - TL:].to_broadcast([CELLS, TL, 2]),
                op=OP.mult)
            px_l = stats.tile([CELLS, 2], F32, tag="pxl")
            nc.vector.tensor_reduce(
                out=px_l[:], in_=exl[:].rearrange("p (t c) -> p c t", t=TL, c=2),
                axis=AX.X, op=OP.add)
            pqs_l = stats.tile([CELLS, TL], F32, tag="pql")
            nc.vector.tensor_tensor(out=pqs_l[:], in0=qv_sb[:, T - TL:],
                                    in1=e_all[:, T - TL:], op=OP.mult)

            # den = (acc_es + last_es) + 1 in one two-scalar op
            den = stats.tile([CELLS, 1], F32, tag="den")
            nc.vector.tensor_scalar(den[:], acc_es[:], last_es[:], 1.0,
                                    OP.add, OP.add)
            rec = stats.tile([CELLS, 1], F32, tag="rec")
            nc.vector.reciprocal(rec[:], den[:])
            ra = stats.tile([CELLS, 1], F32, tag="ra")
            nc.vector.tensor_scalar_mul(ra[:], rec[:], act_sb[:])

            # pooled stays UNNORMALIZED; 1/den folds into the epilogue scale
            pooled = single.tile([CELLS, IN_DIM], F32)
            nc.vector.tensor_tensor(out=pooled[:, 2:2 + GENE_DIM],
                                    in0=acc_pg[:], in1=last_pg[:], op=OP.add)
            # cs = cent * (den - 1) = cent*den - cent, one fused op
            cs = stats.tile([CELLS, 2], F32, tag="cs")
            nc.vector.scalar_tensor_tensor(out=cs[:], in0=cent_sb[:],
                                           scalar=den[:], in1=cent_sb[:],
                                           op0=OP.mult, op1=OP.subtract)
            sxx = stats.tile([CELLS, 2], F32, tag="sxx")
            nc.vector.tensor_tensor(out=sxx[:], in0=px_m[:], in1=px_l[:],
                                    op=OP.add)
            nc.vector.tensor_tensor(out=pooled[:, 0:2], in0=sxx[:], in1=cs[:],
                                    op=OP.subtract)
            pq_l = stats.tile([CELLS, 1], F32, tag="pqls")
            nc.vector.tensor_reduce(out=pq_l[:], in_=pqs_l[:],
                                    axis=AX.X, op=OP.add)
            nc.vector.tensor_tensor(out=pooled[:, 34:35], in0=pq_m[:],
                                    in1=pq_l[:], op=OP.add)

            psum_t = psum.tile([128, 128], F32, tag="pt")
            nc.tensor.transpose(out=psum_t[:IN_DIM, :CELLS], in_=pooled[:],
                                identity=ident[:])
            lhsT = single.tile([128, CELLS], F32)
            nc.scalar.copy(lhsT[:IN_DIM, :], psum_t[:IN_DIM, :CELLS])

            out_ps = psum.tile([128, D], F32, tag="out")
            nc.tensor.matmul(out=out_ps[:], lhsT=lhsT[:IN_DIM, :],
                             rhs=wrhs_sb[:IN_DIM, :], start=True, stop=True)
            out_sb = work.tile([CELLS, D], F32, tag="outs")
            nc.vector.scalar_tensor_tensor(out=out_sb[:], in0=out_ps[:],
                                           scalar=ra[:], in1=bact[:],
                                           op0=OP.mult, op1=OP.add)
            nc.sync.dma_start(out=out_d.ap(), in_=out_sb[:])

            if dbg:
                nc.sync.dma_start(out=dbg_tensors["d_e"].ap(), in_=e_all[:])
                nc.sync.dma_start(out=dbg_tensors["d_spre"].ap(), in_=spre[:])
                nc.sync.dma_start(out=dbg_tensors["d_pooled"].ap(), in_=pooled[:])
                den_pad = single.tile([CELLS, 1], F32)
                nc.vector.tensor_copy(den_pad[:], den[:])
                nc.sync.dma_start(out=dbg_tensors["d_den"].ap(), in_=den_pad[:])
                nc.sync.dma_start(out=dbg_tensors["d_lhsT"].ap()[:IN_DIM, :],
                                  in_=lhsT[:IN_DIM, :])
                nc.sync.dma_start(out=dbg_tensors["d_g0"].ap(), in_=g_t[0][:])

    nc.compile()
    return nc


def host_prep(omics_x, centroids, omics_gene_ids, omics_qv, omics_valid_mask,
              query_valid_mask, gene_emb, W_gat, a_src, a_dst, b_gat):
    f32 = np.float32
    wa = (W_gat.astype(np.float64) @ a_src.astype(np.float64)).astype(f32)  # [35]
    gs = (gene_emb.astype(f32) @ wa[2:2 + GENE_DIM]).astype(f32)  # [VOCAB]
    table = np.zeros((GENE_VOCAB, F), f32)  # [VOCAB, 64] (256B rows)
    table[:, 0] = gs
    table[:, 1:1 + GENE_DIM] = gene_emb.astype(f32)

    consts = np.zeros((CELLS, 516), f32)
    consts[:, 0:256] = np.tile(wa[0:2], T)[None, :]
    consts[:, 256:512] = b_gat.astype(f32)[None, :]
    consts[:, 512] = wa[34]  # runtime activation scale: no recompile on
    # different parameter values
    wrhs = np.ascontiguousarray(W_gat.astype(f32))  # [35, 256]
    ident = np.eye(128, dtype=f32)

    NC_TOT = B * Q
    x = np.ascontiguousarray(omics_x.astype(f32).reshape(NC_TOT, T * 2))
    qv = np.ascontiguousarray(omics_qv.astype(f32).reshape(NC_TOT, T))
    ids = omics_gene_ids.astype(np.int16).reshape(NC_TOT, T)
    cent = np.ascontiguousarray(centroids.astype(f32).reshape(NC_TOT, 2))
    validf = omics_valid_mask.reshape(NC_TOT, T).astype(f32)
    madd = np.ascontiguousarray((validf - 1.0) * f32(1e30))
    active = (query_valid_mask.reshape(NC_TOT).astype(bool)
              & omics_valid_mask.reshape(NC_TOT, T).astype(bool).any(-1))
    active = np.ascontiguousarray(active.astype(f32).reshape(NC_TOT, 1))

    in_maps = []
    for c in range(N_CORES):
        sl = slice(c * CELLS, (c + 1) * CELLS)
        # flat gather index i = t*CELLS + cell -> dst[cell, t]
        flat = ids[sl].T.reshape(-1)  # flat[i] = ids[i % 128, i // 128]
        wrapped = np.ascontiguousarray(
            np.tile(flat.reshape(-1, 16).T, (8, 1)))  # [128, T*CELLS/16]
        in_maps.append({
            "x_in": x[sl], "qv_in": qv[sl], "madd_in": madd[sl],
            "idx_in": wrapped, "cent_in": cent[sl], "act_in": active[sl],
            "consts_in": consts, "wrhs_in": wrhs, "table_in": table,
            "ident_in": ident,
        })
    return in_maps, float(wa[34])


def _get_program():
    # the program is fully parameter-independent: one compile, ever
    if "prog" not in _CACHE:
        _CACHE["prog"] = build_program()
    return _CACHE["prog"]


def kernel(omics_x, centroids, omics_gene_ids, omics_qv, omics_valid_mask,
           query_valid_mask, gene_emb, W_gat, a_src, a_dst, b_gat,
           trace=False):
    in_maps, wa34 = host_prep(
        np.asarray(omics_x), np.asarray(centroids), np.asarray(omics_gene_ids),
        np.asarray(omics_qv), np.asarray(omics_valid_mask),
        np.asarray(query_valid_mask), np.asarray(gene_emb), np.asarray(W_gat),
        np.asarray(a_src), np.asarray(a_dst), np.asarray(b_gat))
    nc = _get_program()
    res = run_bass_kernel_spmd(nc, in_maps, core_ids=list(range(N_CORES)),
                               trace=trace)
    global LAST_RESULTS
    LAST_RESULTS = res
    outs = [res.results[c]["out"] for c in range(N_CORES)]
    full = np.concatenate(outs, axis=0).reshape(B, Q, D)
    return full.astype(np.float32)

